# revision 1
# baseline (speedup 1.0000x reference)
"""Trainium2 Bass kernel for nn_DILATELoss (soft-DTW loss_shape value).

Takes FULL unsharded inputs {pred:[4096,1] f32, target:[4096,1] f32},
returns the FULL output (float32 scalar) = soft-DTW(D, gamma=0.01),
D[i,j] = (target_i - pred_j)^2, matching reference.py.

The DP is a serial anti-diagonal wavefront (8191 diagonals) — the
sharding hint notes intra-series splits need per-diagonal halo exchange,
so the kernel is replicated SPMD on all 8 cores (identical inputs) and
the result is read from core 0.

Per-core algorithm (see work/dilate_v2.py devloop copy):
  Lazy-log pair state per diagonal k: (B, S) with R = B - gamma*ln(S):
    m     = min(B1[q-1], B1[q], B2[q-1])
    E_c   = exp(-(B_c - m)/gamma)            (one ACT op, 3 chunks)
    S_new = E_a*S2[q-1] + E_b*S1[q-1] + E_c*S1[q]
    B_new = d_k + m
  which evaluates the exact softmin recurrence while keeping the serial
  cross-step chain off the Activation engine; S is folded into B every
  Q=32 steps to bound exponent drift (|B-R| <= Q*gamma*ln3 ~ 0.35).

  Layout: lane i = 32p - 8 + q (partition p, slot q of 40): each
  partition redundantly shadows the previous partition's top 8 lanes, so
  a superstep of 8 diagonals needs no cross-partition access; halos are
  refreshed once per superstep with a PE shift-matmul (+PSUM->SBUF copy).

  D diagonals are bulk-generated per 256-diagonal window from a
  reversed padded pred vector (positive-stride skewed DMA + DVE sub +
  ACT square), double-buffered against the DP sweep.

  The whole sweep runs as a tc.For_i loop whose body covers 2 windows
  (512 steps); only the window-DMA source offset is loop-dependent.
"""

import sys

sys.path.insert(0, "/opt/trn_rl_repo")

from contextlib import ExitStack

import numpy as np

import concourse.bass as bass  # noqa: F401
import concourse.tile as tile
from concourse import mybir
from concourse.ap import AP

GAMMA = 0.01
BIG = 1e8
F32 = mybir.dt.float32

N = 4096
P = 128
S_STEP = 8
Q_SYNC = 32
WIN = 256
N_CORES = 8


def _host_inputs(pred, target):
    s = S_STEP
    F = N // P
    H = s + F
    t = np.ascontiguousarray(np.asarray(target).reshape(-1)).astype(np.float32)
    p = np.ascontiguousarray(np.asarray(pred).reshape(-1)).astype(np.float32)
    prpad = np.zeros(3 * N, np.float32)
    prpad[N:2 * N] = p
    prrev = np.zeros((1, 4 * N), np.float32)
    prrev[0, N:] = prpad[::-1]
    lane = np.arange(P)[:, None] * F - s + np.arange(H)[None, :]
    tpad = np.where((lane >= 0) & (lane < N),
                    t[np.clip(lane, 0, N - 1)], 0.0).astype(np.float32)
    shiftmat = np.zeros((128, 128), np.float32)
    for k in range(127):
        shiftmat[k, k + 1] = 1.0  # out[m] = in[m-1]
    return {"prrev": prrev, "tpad": tpad, "shiftmat": shiftmat}


def build_tile(tc, outs, ins, s=S_STEP, Q=Q_SYNC, W=WIN, gen_chunks=8,
               iters_limit=None):
    nc = tc.nc
    F = N // P
    H = s + F
    w = H - 1
    nsteps = 2 * N - 1
    body_steps = 2 * W
    iters = (nsteps + body_steps - 1) // body_steps
    if iters_limit is not None:
        iters = iters_limit
    assert body_steps % Q == 0 and body_steps % s == 0

    prrev_ap, tpad_ap, shiftmat_ap = ins
    prrev_t = prrev_ap.tensor
    out_ap = outs[0]

    with ExitStack() as ctx:
        const_pool = ctx.enter_context(tc.tile_pool(name="const", bufs=1))
        state_pool = ctx.enter_context(tc.tile_pool(name="state", bufs=1))
        m_pool = ctx.enter_context(tc.tile_pool(name="m", bufs=3))
        t_pool = ctx.enter_context(tc.tile_pool(name="T", bufs=3))
        e_pool = ctx.enter_context(tc.tile_pool(name="E", bufs=3))
        u_pool = ctx.enter_context(tc.tile_pool(name="U", bufs=3))
        ps_pool = ctx.enter_context(tc.tile_pool(name="ps", bufs=4, space="PSUM"))
        fin_pool = ctx.enter_context(tc.tile_pool(name="fin", bufs=1))

        tpad_sb = const_pool.tile([P, H], F32, tag="tpad")
        nc.sync.dma_start(tpad_sb[:], tpad_ap[:])
        shift_sb = const_pool.tile([128, 128], F32, tag="shift")
        nc.sync.dma_start(shift_sb[:], shiftmat_ap[:])

        Bt = [state_pool.tile([P, H], F32, tag=f"B{i}", name=f"B{i}") for i in range(2)]
        St = [state_pool.tile([P, H], F32, tag=f"S{i}", name=f"S{i}") for i in range(2)]
        dwX = state_pool.tile([P, H, W], F32, tag="dwX", name="dwX")
        dwY = state_pool.tile([P, H, W], F32, tag="dwY", name="dwY")
        nc.vector.memset(Bt[0][:], BIG)
        nc.vector.memset(Bt[1][:], BIG)
        nc.vector.memset(St[0][:], 1.0)
        nc.vector.memset(St[1][:], 1.0)
        # virtual origin R[-1,-1]=0: step k=0 reads a = B2[(p=0, q=s-1)]
        nc.vector.memset(Bt[0][0:1, s - 1:s], 0.0)

        wc = W // gen_chunks

        def gen_window(dw, base):
            # dw[p, q, kkr] = prrev[base + F*p + q + kkr]
            for c in range(gen_chunks):
                ap_src = AP(prrev_t, base + c * wc, [[F, P], [1, H], [1, wc]])
                nc.sync.dma_start(dw[:, :, c * wc:(c + 1) * wc], ap_src)
                cs = slice(c * wc, (c + 1) * wc)
                tb = tpad_sb[:].unsqueeze(2).broadcast_to([P, H, wc])
                nc.vector.tensor_tensor(dw[:, :, cs], dw[:, :, cs], tb,
                                        mybir.AluOpType.subtract)
                nc.scalar.activation(dw[:, :, cs], dw[:, :, cs],
                                     mybir.ActivationFunctionType.Square)

        def base_for(v):
            return 3 * N - v * W - W - s

        state = {"cur": 1, "prev": 0}

        def step(kk_body, dw, kk):
            dk = dw[:, 1:H, W - 1 - kk]
            Bc, Sc = Bt[state["cur"]], St[state["cur"]]
            Bp, Sp = Bt[state["prev"]], St[state["prev"]]
            m = m_pool.tile([P, w], F32, tag="m", name=f"m{kk_body}")
            mbc = m_pool.tile([P, w], F32, tag="mbc", name=f"mbc{kk_body}")
            T = t_pool.tile([P, 3 * w], F32, tag="T", name=f"T{kk_body}")
            E = e_pool.tile([P, 3 * w], F32, tag="E", name=f"E{kk_body}")
            U = u_pool.tile([P, 3 * w], F32, tag="U", name=f"U{kk_body}")

            nc.vector.tensor_tensor(mbc[:], Bc[:, 0:w], Bc[:, 1:H], mybir.AluOpType.min)
            nc.vector.tensor_tensor(m[:], mbc[:], Bp[:, 0:w], mybir.AluOpType.min)
            nc.gpsimd.tensor_tensor(T[:, 0:w], Bp[:, 0:w], m[:], mybir.AluOpType.subtract)
            nc.vector.tensor_tensor(T[:, w:2 * w], Bc[:, 0:w], m[:], mybir.AluOpType.subtract)
            nc.vector.tensor_tensor(T[:, 2 * w:3 * w], Bc[:, 1:H], m[:], mybir.AluOpType.subtract)
            nc.scalar.activation(E[:], T[:], mybir.ActivationFunctionType.Exp,
                                 scale=-1.0 / GAMMA)
            nc.gpsimd.tensor_tensor(U[:, 0:w], E[:, 0:w], Sp[:, 0:w], mybir.AluOpType.mult)
            nc.vector.tensor_tensor(U[:, w:2 * w], E[:, w:2 * w], Sc[:, 0:w], mybir.AluOpType.mult)
            nc.gpsimd.tensor_tensor(U[:, 2 * w:3 * w], E[:, 2 * w:3 * w], Sc[:, 1:H],
                                    mybir.AluOpType.mult)
            u3 = U[:].rearrange("p (c q) -> p q c", c=3, q=w)
            nc.vector.tensor_reduce(Sp[:, 1:H], u3, mybir.AxisListType.X, mybir.AluOpType.add)
            nc.gpsimd.tensor_tensor(Bp[:, 1:H], dk, m[:], mybir.AluOpType.add)

            state["cur"], state["prev"] = state["prev"], state["cur"]

            if (kk_body % Q) == (Q - 1):
                for Bx, Sx in ((Bt[state["cur"]], St[state["cur"]]),
                               (Bt[state["prev"]], St[state["prev"]])):
                    lnt = m_pool.tile([P, w], F32, tag="lnt", name=f"lnt{kk_body}")
                    nc.scalar.activation(lnt[:], Sx[:, 1:H], mybir.ActivationFunctionType.Ln)
                    nc.vector.scalar_tensor_tensor(
                        Bx[:, 1:H], lnt[:], -GAMMA, Bx[:, 1:H],
                        mybir.AluOpType.mult, mybir.AluOpType.add)
                    nc.gpsimd.memset(Sx[:, 1:H], 1.0)

            if (kk_body % s) == (s - 1):
                for X, fill in ((Bt[state["cur"]], BIG), (St[state["cur"]], 1.0),
                                (Bt[state["prev"]], BIG), (St[state["prev"]], 1.0)):
                    ps = ps_pool.tile([128, s], F32, tag="ps", name=f"ps{kk_body}")
                    nc.tensor.matmul(ps[:], shift_sb[:], X[:, F:F + s],
                                     start=True, stop=True)
                    nc.scalar.copy(X[:, 0:s], ps[:])
                    nc.gpsimd.memset(X[0:1, 0:s], fill)

        gen_window(dwX, base_for(0))
        gen_window(dwY, base_for(1))

        def body(i):
            for kk in range(W):
                step(kk, dwX, kk)
            gen_window(dwX, i * (-body_steps) + base_for(2))   # window 2i+2
            for kk in range(W):
                step(W + kk, dwY, kk)
            gen_window(dwY, i * (-body_steps) + base_for(3))   # window 2i+3

        with tc.For_i(0, iters, 1) as i:
            body(i)

        # iters*body_steps = nsteps+1: one trailing bogus step, which wrote
        # the *other* buffer; the final diagonal lives in Bt/St[prev].
        extra = iters * body_steps - nsteps
        fin = state["cur"] if extra % 2 == 0 else state["prev"]
        Bf, Sf = Bt[fin], St[fin]
        lnf = fin_pool.tile([P, 1], F32, tag="lnf")
        outc = fin_pool.tile([P, 1], F32, tag="outc")
        nc.scalar.activation(lnf[:], Sf[:, H - 1:H], mybir.ActivationFunctionType.Ln)
        nc.vector.scalar_tensor_tensor(outc[:], lnf[:], -GAMMA, Bf[:, H - 1:H],
                                       mybir.AluOpType.mult, mybir.AluOpType.add)
        nc.sync.dma_start(out_ap[0:1, 0:1], outc[127:128, 0:1])


def kernel(pred, target):
    from concourse.bass_test_utils import run_kernel

    hi = _host_inputs(pred, target)
    ins_one = [hi["prrev"], hi["tpad"], hi["shiftmat"]]
    out_like = [np.zeros((1, 1), np.float32)]

    res = run_kernel(
        lambda tc, outs, inaps: build_tile(tc, outs, inaps),
        None,
        [ins_one] * N_CORES,
        output_like=[out_like] * N_CORES,
        bass_type=tile.TileContext,
        check_with_sim=False,
        check_with_hw=True,
        trace_sim=False,
        num_cores=N_CORES,
    )
    val = np.float32(list(res.results[0].values())[0][0, 0])
    return np.asarray(val, dtype=np.float32)


if __name__ == "__main__":
    rng = np.random.default_rng(0)
    pred = rng.standard_normal((N, 1)).astype(np.float32)
    target = rng.standard_normal((N, 1)).astype(np.float32)
    print(kernel(pred=pred, target=target))

